# revision 4
# baseline (speedup 1.0000x reference)
"""Trainium2 Bass kernel for per-edge dot products (DGL u_dot_v / DotPredictor).

score[e] = sum_d h[src[e], d] * h[dst[e], d]

Strategy (v2 — SBUF-resident gather):
  - Split the E=6.4M edges evenly across 8 NeuronCores (800k each).
  - The node table h (100000x128 fp32) is transposed on the host to
    hT[seg, feature, node] and split into NSEG=5 segments of 20000 nodes
    (80KB/partition each); two segments (src-side, dst-side) are resident in
    SBUF at a time. Edges are bucketed on the host into 25 (src_seg, dst_seg)
    buckets, visited in snake order so each step swaps only one segment.
  - Per 2048-edge chunk: GPSIMD ap_gather pulls the src and dst embedding
    COLUMNS ([128, 2048] fp32) straight out of the SBUF-resident segments
    (no HBM traffic, no DMA descriptors), DVE multiplies elementwise, and the
    PE reduces across the 128 feature partitions via a ones[128,1] matmul
    into PSUM [1, 2048]. Scores DMA from PSUM to HBM.
  - Host unpermutes scores back to the original edge order; bucket-overflow
    edges (beyond the static pad) fall back to a host dot product.
"""
import sys

sys.path.insert(0, "/opt/trn_rl_repo")

import numpy as np

import concourse.bacc as bacc
import concourse.bass as bass
import concourse.mybir as mybir
import concourse.tile as tile
from concourse.bass_utils import run_bass_kernel_spmd

# Problem shape (hardcoded per contract).
N, E, D = 100000, 6400000, 128
M = 8                      # NeuronCores
P = 128                    # SBUF partitions
E_PER = E // M             # 800000 edges per core
NSEG = 5                   # node-table segments (two fit in SBUF at once)
S = N // NSEG              # 20000 rows per segment
NB = NSEG * NSEG           # 25 (src_seg, dst_seg) buckets
C = 2048                   # edges per chunk (= ap_gather num_idxs)
NCPB = 17                  # chunks per bucket
B_PAD = NCPB * C           # 34816 padded bucket size (mean 32000 + ~15 sigma)
TC = NB * NCPB             # 425 chunks per core
IW = C // 16               # idx columns per chunk (wrapped layout)
NMM = C // 512             # matmuls per chunk (PSUM moving-free limit 512)


def _snake():
    """Bucket visit order: (a, b) pairs where consecutive steps change only
    one segment."""
    order = []
    for a in range(NSEG):
        bs = range(NSEG) if a % 2 == 0 else range(NSEG - 1, -1, -1)
        for b in bs:
            order.append((a, b))
    return order


SNAKE = _snake()


def build_nc():
    nc = bacc.Bacc("TRN2", target_bir_lowering=False, debug=False)
    f32 = mybir.dt.float32
    hT = nc.dram_tensor("hT", [NSEG, P, S], f32, kind="ExternalInput")
    sidx = nc.dram_tensor("sidx", [TC, P, IW], mybir.dt.int16, kind="ExternalInput")
    didx = nc.dram_tensor("didx", [TC, P, IW], mybir.dt.int16, kind="ExternalInput")
    out = nc.dram_tensor("out", [TC, 1, C], f32, kind="ExternalOutput")

    with tile.TileContext(nc) as tc:
        with (
            tc.tile_pool(name="seg", bufs=1) as seg_pool,
            tc.tile_pool(name="ones", bufs=1) as ones_pool,
            tc.tile_pool(name="idx", bufs=3) as idx_pool,
            tc.tile_pool(name="cols", bufs=2) as col_pool,
            tc.tile_pool(name="score", bufs=2) as score_pool,
            tc.psum_pool(name="ps", bufs=2) as ps_pool,
        ):
            ones = ones_pool.tile([P, 1], f32)
            nc.vector.memset(ones[:], 1.0)
            prev_a = prev_b = None
            seg_a = seg_b = None
            for k, (a, b) in enumerate(SNAKE):
                if a != prev_a:
                    seg_a = seg_pool.tile([P, S], f32, tag="a")
                    nc.sync.dma_start(out=seg_a[:], in_=hT[a])
                    prev_a = a
                if b != prev_b:
                    seg_b = seg_pool.tile([P, S], f32, tag="b")
                    nc.sync.dma_start(out=seg_b[:], in_=hT[b])
                    prev_b = b
                for j in range(NCPB):
                    c = k * NCPB + j
                    idx_s = idx_pool.tile([P, IW], mybir.dt.int16, tag="s")
                    nc.sync.dma_start(out=idx_s[:], in_=sidx[c])
                    idx_d = idx_pool.tile([P, IW], mybir.dt.int16, tag="d")
                    nc.sync.dma_start(out=idx_d[:], in_=didx[c])
                    s_cols = col_pool.tile([P, C], f32, tag="s")
                    d_cols = col_pool.tile([P, C], f32, tag="d")
                    nc.gpsimd.ap_gather(
                        s_cols[:].rearrange("p (n d) -> p n d", d=1),
                        seg_a[:].rearrange("p (n d) -> p n d", d=1),
                        idx_s[:],
                        P, S, 1, C,
                    )
                    nc.gpsimd.ap_gather(
                        d_cols[:].rearrange("p (n d) -> p n d", d=1),
                        seg_b[:].rearrange("p (n d) -> p n d", d=1),
                        idx_d[:],
                        P, S, 1, C,
                    )
                    nc.vector.tensor_tensor(
                        out=s_cols[:],
                        in0=s_cols[:],
                        in1=d_cols[:],
                        op=mybir.AluOpType.mult,
                    )
                    ps = ps_pool.tile([P, C], f32, tag="ps")
                    for q in range(NMM):
                        nc.tensor.matmul(
                            ps[0:1, q * 512 : (q + 1) * 512],
                            ones[:],
                            s_cols[:, q * 512 : (q + 1) * 512],
                            start=True,
                            stop=True,
                        )
                    sc = score_pool.tile([1, C], f32, tag="sc")
                    nc.scalar.copy(sc[:], ps[0:1, :])
                    nc.sync.dma_start(out=out[c], in_=sc[:])
    nc.compile()
    return nc


_NC_CACHE = None


def _get_nc():
    global _NC_CACHE
    if _NC_CACHE is None:
        _NC_CACHE = build_nc()
    return _NC_CACHE


_STEP_OF = {ab: k for k, ab in enumerate(SNAKE)}


def _prep_core(src_c, dst_c):
    """Bucket one core's edges. Returns (sidx, didx, pos, keep) where
    sidx/didx are the wrapped [TC, P, IW] int16 device index tensors, pos is
    each kept edge's flat position in the bucketed stream, keep the mask."""
    sa = (src_c // S).astype(np.int64)
    sb = (dst_c // S).astype(np.int64)
    step_lut = np.empty((NSEG, NSEG), dtype=np.int64)
    for (a, b), k in _STEP_OF.items():
        step_lut[a, b] = k
    bkt = step_lut[sa, sb]

    # rank of each edge within its bucket, in original order (stable sort)
    order = np.argsort(bkt, kind="stable")
    counts = np.bincount(bkt, minlength=NB)
    starts = np.concatenate([[0], np.cumsum(counts)[:-1]])
    rank = np.empty(E_PER, dtype=np.int64)
    rank[order] = np.arange(E_PER, dtype=np.int64) - starts[bkt[order]]
    keep = rank < B_PAD
    pos = bkt * B_PAD + rank  # valid where keep

    spad = np.zeros(NB * B_PAD, dtype=np.int16)
    dpad = np.zeros(NB * B_PAD, dtype=np.int16)
    kp = pos[keep]
    spad[kp] = (src_c[keep] % S).astype(np.int16)
    dpad[kp] = (dst_c[keep] % S).astype(np.int16)

    def wrap(arr):
        # [NB*B_PAD] -> [TC, IW, 16] -> wrapped [TC, 16, IW] -> tiled [TC, P, IW]
        a = arr.reshape(TC, IW, 16).transpose(0, 2, 1)
        return np.ascontiguousarray(np.tile(a, (1, P // 16, 1)))

    return wrap(spad), wrap(dpad), pos, keep


def _transpose_table(h):
    # h [N, D] fp32 -> hT [NSEG, P, S]: hT[g, p, i] = h[g*S + i, p]
    return np.ascontiguousarray(h.T.reshape(P, NSEG, S).transpose(1, 0, 2))


def build_in_maps(inputs):
    """Host-side staging: shard edges, bucket, wrap indices, transpose h.
    Returns (in_maps, metas)."""
    h = np.ascontiguousarray(np.asarray(inputs["h"], dtype=np.float32))
    src = np.ascontiguousarray(np.asarray(inputs["src"]).astype(np.int32))
    dst = np.ascontiguousarray(np.asarray(inputs["dst"]).astype(np.int32))
    assert h.shape == (N, D) and src.shape == (E,) and dst.shape == (E,)

    hT = _transpose_table(h)
    in_maps = []
    metas = []
    for i in range(M):
        sl = slice(i * E_PER, (i + 1) * E_PER)
        sidx, didx, pos, keep = _prep_core(src[sl], dst[sl])
        in_maps.append({"hT": hT, "sidx": sidx, "didx": didx})
        metas.append((pos, keep))
    return in_maps, metas


def run(inputs, trace=False, trace_kwargs=None):
    """Shard, execute on 8 cores, gather. Returns (scores[E] fp32, results)."""
    h = np.asarray(inputs["h"], dtype=np.float32)
    src = np.asarray(inputs["src"]).astype(np.int32)
    dst = np.asarray(inputs["dst"]).astype(np.int32)
    in_maps, metas = build_in_maps(inputs)

    try:
        res = run_bass_kernel_spmd(
            _get_nc(),
            in_maps,
            core_ids=list(range(M)),
            trace=trace,
            trace_kwargs=trace_kwargs or {},
        )
    except ModuleNotFoundError:
        # axon build without NTFF profiling hooks — run without trace
        res = run_bass_kernel_spmd(
            _get_nc(), in_maps, core_ids=list(range(M)), trace=False
        )

    scores = np.empty(E, dtype=np.float32)
    for i in range(M):
        sl = slice(i * E_PER, (i + 1) * E_PER)
        pos, keep = metas[i]
        flat = np.asarray(res.results[i]["out"], dtype=np.float32).reshape(-1)
        sc = np.empty(E_PER, dtype=np.float32)
        sc[keep] = flat[pos[keep]]
        if not keep.all():  # host fallback for bucket-overflow edges
            ov = ~keep
            sc[ov] = np.einsum(
                "ed,ed->e", h[src[sl][ov]], h[dst[sl][ov]]
            ).astype(np.float32)
        scores[sl] = sc
    return scores, res


def kernel(**inputs) -> np.ndarray:
    return run(inputs)[0]


# revision 6
# speedup vs baseline: 23.4868x; 23.4868x over previous
"""Trainium2 Bass kernel for per-edge dot products (DGL u_dot_v / DotPredictor).

score[e] = sum_d h[src[e], d] * h[dst[e], d]

Strategy:
  - Split the E=6.4M edges evenly across 8 NeuronCores (800k each); replicate
    the node table h (100000x128 fp32, 51MB) in each core's HBM.
  - Bulk row gather uses the GPSIMD ucode `dma_gather` (InstDMAGatherAnt):
    thousands of 512B row fetches per instruction. Its indices are int16, so
    the node table is viewed as 4 segments of 25000 rows and each core's edges
    are bucketed on the host into 16 (src_seg, dst_seg) buckets (padded to a
    fixed size so the SPMD program is static). Edges past the pad (won't
    happen for the expected distribution) fall back to a host dot product.
  - Per 5120-edge chunk: gather h[src] and h[dst] rows to SBUF, multiply +
    per-row reduce on the vector engine, stream 1 score/edge back to HBM.
  - Host unpermutes scores back to the original edge order.
"""
import sys

sys.path.insert(0, "/opt/trn_rl_repo")

import numpy as np

import concourse.bacc as bacc
import concourse.bass as bass
import concourse.mybir as mybir
import concourse.tile as tile
from concourse.bass_utils import run_bass_kernel_spmd

# Problem shape (hardcoded per contract).
N, E, D = 100000, 6400000, 128
M = 8                      # NeuronCores
P = 128                    # SBUF partitions
E_PER = E // M             # 800000 edges per core
NSEG = 4                   # node-table segments (int16 index range)
S = N // NSEG              # 25000 rows per segment
NBUCKET = NSEG * NSEG      # 16 (src_seg, dst_seg) buckets
G = 5120                   # indices per dma_gather
CPG = G // P               # 50 dst columns per gather
B_PAD = 51200              # padded bucket size (10 chunks of G)
NCPB = B_PAD // G          # 8 chunks per bucket
TC = NBUCKET * NCPB        # 128 chunks per core
IW = G // 16               # idx columns per chunk (wrapped layout)
SCRATCH = 16384            # SWDGE descriptor-ring carveout bytes
SINGLE_PACKET = False      # one giant packet overflows the SWDGE ring; use
                           # multi-packet mode so the ucode reclaims space
NQUEUES = 4                # SWDGE queues: parallel Q7 descriptor generation


def build_nc():
    nc = bacc.Bacc(
        "TRN2",
        target_bir_lowering=False,
        debug=False,
        dynamic_dma_scratch_size=SCRATCH,
        num_swdge_queues=NQUEUES,
    )
    h = nc.dram_tensor("h", [N, D], mybir.dt.float32, kind="ExternalInput")
    sidx = nc.dram_tensor("sidx", [TC, P, IW], mybir.dt.int16, kind="ExternalInput")
    didx = nc.dram_tensor("didx", [TC, P, IW], mybir.dt.int16, kind="ExternalInput")
    out = nc.dram_tensor("out", [TC, P, CPG], mybir.dt.float32, kind="ExternalOutput")

    with tile.TileContext(nc) as tc:
        with (
            tc.tile_pool(name="idx", bufs=2) as idx_pool,
            tc.tile_pool(name="rows", bufs=4) as row_pool,
            tc.tile_pool(name="score", bufs=2) as score_pool,
        ):
            for c in range(TC):
                k = c // NCPB
                a, b = k // NSEG, k % NSEG
                idx_s = idx_pool.tile([P, IW], mybir.dt.int16, tag="s")
                idx_d = idx_pool.tile([P, IW], mybir.dt.int16, tag="d")
                nc.sync.dma_start(out=idx_s[:], in_=sidx[c])
                nc.sync.dma_start(out=idx_d[:], in_=didx[c])
                s_rows = row_pool.tile([P, CPG * D], mybir.dt.float32, tag="s")
                d_rows = row_pool.tile([P, CPG * D], mybir.dt.float32, tag="d")
                nc.gpsimd.dma_gather(
                    s_rows[:].rearrange("p (c d) -> p c d", d=D),
                    h[a * S : (a + 1) * S, :],
                    idx_s[:],
                    G,
                    G,
                    D,
                    single_packet=SINGLE_PACKET,
                    queue_num=(2 * c) % NQUEUES,
                )
                nc.gpsimd.dma_gather(
                    d_rows[:].rearrange("p (c d) -> p c d", d=D),
                    h[b * S : (b + 1) * S, :],
                    idx_d[:],
                    G,
                    G,
                    D,
                    single_packet=SINGLE_PACKET,
                    queue_num=(2 * c + 1) % NQUEUES,
                )
                nc.vector.tensor_tensor(
                    out=s_rows[:],
                    in0=s_rows[:],
                    in1=d_rows[:],
                    op=mybir.AluOpType.mult,
                )
                score = score_pool.tile([P, CPG], mybir.dt.float32, tag="sc")
                nc.vector.tensor_reduce(
                    out=score[:],
                    in_=s_rows[:].rearrange("p (c d) -> p c d", d=D),
                    axis=mybir.AxisListType.X,
                    op=mybir.AluOpType.add,
                )
                nc.sync.dma_start(out=out[c], in_=score[:])
    nc.compile()
    return nc


_NC_CACHE = None


def _get_nc():
    global _NC_CACHE
    if _NC_CACHE is None:
        _NC_CACHE = build_nc()
    return _NC_CACHE


def _prep_core(src_c, dst_c):
    """Bucket one core's edges. Returns (sidx, didx, pos, keep) where
    sidx/didx are the wrapped [TC, P, IW] int16 device index tensors, pos is
    each kept edge's flat position in the bucketed stream, keep the mask."""
    b = (src_c // S).astype(np.int32) * NSEG + (dst_c // S).astype(np.int32)
    # rank of each edge within its bucket, in original order
    rank = np.empty(E_PER, dtype=np.int64)
    for k in range(NBUCKET):
        m = b == k
        rank[m] = np.arange(m.sum(), dtype=np.int64)
    keep = rank < B_PAD
    pos = b.astype(np.int64) * B_PAD + rank  # valid where keep

    spad = np.zeros(NBUCKET * B_PAD, dtype=np.int16)
    dpad = np.zeros(NBUCKET * B_PAD, dtype=np.int16)
    kp = pos[keep]
    spad[kp] = (src_c[keep] % S).astype(np.int16)
    dpad[kp] = (dst_c[keep] % S).astype(np.int16)

    def wrap(arr):
        # [NBUCKET*B_PAD] -> [TC, G] -> wrapped [TC, 16, IW] -> tiled [TC, P, IW]
        a = arr.reshape(TC, IW, 16).transpose(0, 2, 1)
        return np.ascontiguousarray(np.tile(a, (1, P // 16, 1)))

    return wrap(spad), wrap(dpad), pos, keep


def build_in_maps(inputs):
    """Host-side staging: shard edges, bucket, wrap indices.
    Returns (in_maps, metas)."""
    h = np.ascontiguousarray(np.asarray(inputs["h"], dtype=np.float32))
    src = np.ascontiguousarray(np.asarray(inputs["src"]).astype(np.int32))
    dst = np.ascontiguousarray(np.asarray(inputs["dst"]).astype(np.int32))
    assert h.shape == (N, D) and src.shape == (E,) and dst.shape == (E,)

    in_maps = []
    metas = []
    for i in range(M):
        sl = slice(i * E_PER, (i + 1) * E_PER)
        sidx, didx, pos, keep = _prep_core(src[sl], dst[sl])
        in_maps.append({"h": h, "sidx": sidx, "didx": didx})
        metas.append((pos, keep))
    return in_maps, metas


def run(inputs, trace=False, trace_kwargs=None):
    """Shard, execute on 8 cores, gather. Returns (scores[E] fp32, results)."""
    h = np.asarray(inputs["h"], dtype=np.float32)
    src = np.asarray(inputs["src"]).astype(np.int32)
    dst = np.asarray(inputs["dst"]).astype(np.int32)
    in_maps, metas = build_in_maps(inputs)

    try:
        res = run_bass_kernel_spmd(
            _get_nc(),
            in_maps,
            core_ids=list(range(M)),
            trace=trace,
            trace_kwargs=trace_kwargs or {},
        )
    except ModuleNotFoundError:
        # axon build without NTFF profiling hooks — run without trace
        res = run_bass_kernel_spmd(
            _get_nc(), in_maps, core_ids=list(range(M)), trace=False
        )

    scores = np.empty(E, dtype=np.float32)
    for i in range(M):
        sl = slice(i * E_PER, (i + 1) * E_PER)
        pos, keep = metas[i]
        out_arr = np.asarray(res.results[i]["out"], dtype=np.float32)
        # out_arr[c, p, j] is the score of bucketed position c*G + j*128 + p
        flat = out_arr.transpose(0, 2, 1).reshape(-1)
        sc = np.empty(E_PER, dtype=np.float32)
        sc[keep] = flat[pos[keep]]
        if not keep.all():  # host fallback for bucket-overflow edges
            ov = ~keep
            sc[ov] = np.einsum(
                "ed,ed->e", h[src[sl][ov]], h[dst[sl][ov]]
            ).astype(np.float32)
        scores[sl] = sc
    return scores, res


def kernel(**inputs) -> np.ndarray:
    return run(inputs)[0]



# revision 7
# speedup vs baseline: 26.4599x; 1.1266x over previous
"""Trainium2 Bass kernel for per-edge dot products (DGL u_dot_v / DotPredictor).

score[e] = sum_d h[src[e], d] * h[dst[e], d]

Strategy:
  - Split the E=6.4M edges evenly across 8 NeuronCores (800k each); replicate
    the node table h (100000x128 fp32, 51MB) in each core's HBM.
  - Bulk row gather uses the GPSIMD ucode `dma_gather` (InstDMAGatherAnt):
    thousands of 512B row fetches per instruction. Its indices are int16, so
    the node table is viewed as 4 segments of 25000 rows and each core's edges
    are bucketed on the host into 16 (src_seg, dst_seg) buckets (padded to a
    fixed size so the SPMD program is static). Edges past the pad (won't
    happen for the expected distribution) fall back to a host dot product.
  - Per 5120-edge chunk: gather h[src] and h[dst] rows to SBUF, multiply +
    per-row reduce on the vector engine, stream 1 score/edge back to HBM.
  - Host unpermutes scores back to the original edge order.
"""
import sys

sys.path.insert(0, "/opt/trn_rl_repo")

import numpy as np

import concourse.bacc as bacc
import concourse.bass as bass
import concourse.mybir as mybir
import concourse.tile as tile
from concourse.bass_utils import run_bass_kernel_spmd

# Problem shape (hardcoded per contract).
N, E, D = 100000, 6400000, 128
M = 8                      # NeuronCores
P = 128                    # SBUF partitions
E_PER = E // M             # 800000 edges per core
NSEG = 4                   # node-table segments (int16 index range)
S = N // NSEG              # 25000 rows per segment
NBUCKET = NSEG * NSEG      # 16 (src_seg, dst_seg) buckets
G = 5120                   # indices per dma_gather
CPG = G // P               # 50 dst columns per gather
B_PAD = 51200              # padded bucket size (10 chunks of G)
NCPB = B_PAD // G          # 8 chunks per bucket
TC = NBUCKET * NCPB        # 128 chunks per core
IW = G // 16               # idx columns per chunk (wrapped layout)
SCRATCH = 16384            # SWDGE descriptor-ring carveout bytes
SINGLE_PACKET = False      # one giant packet overflows the SWDGE ring; use
                           # multi-packet mode so the ucode reclaims space
NQUEUES = 4                # SWDGE queues: parallel Q7 descriptor generation


def build_nc():
    nc = bacc.Bacc(
        "TRN2",
        target_bir_lowering=False,
        debug=False,
        dynamic_dma_scratch_size=SCRATCH,
        num_swdge_queues=NQUEUES,
    )
    h = nc.dram_tensor("h", [N, D], mybir.dt.float16, kind="ExternalInput")
    sidx = nc.dram_tensor("sidx", [TC, P, IW], mybir.dt.int16, kind="ExternalInput")
    didx = nc.dram_tensor("didx", [TC, P, IW], mybir.dt.int16, kind="ExternalInput")
    out = nc.dram_tensor("out", [TC, P, CPG], mybir.dt.float32, kind="ExternalOutput")

    with tile.TileContext(nc) as tc:
        with (
            tc.tile_pool(name="idx", bufs=2) as idx_pool,
            tc.tile_pool(name="rows", bufs=4) as row_pool,
            tc.tile_pool(name="score", bufs=2) as score_pool,
        ):
            for c in range(TC):
                k = c // NCPB
                a, b = k // NSEG, k % NSEG
                idx_s = idx_pool.tile([P, IW], mybir.dt.int16, tag="s")
                idx_d = idx_pool.tile([P, IW], mybir.dt.int16, tag="d")
                nc.sync.dma_start(out=idx_s[:], in_=sidx[c])
                nc.sync.dma_start(out=idx_d[:], in_=didx[c])
                s_rows = row_pool.tile([P, CPG * D], mybir.dt.float16, tag="s")
                d_rows = row_pool.tile([P, CPG * D], mybir.dt.float16, tag="d")
                nc.gpsimd.dma_gather(
                    s_rows[:].rearrange("p (c d) -> p c d", d=D),
                    h[a * S : (a + 1) * S, :],
                    idx_s[:],
                    G,
                    G,
                    D,
                    single_packet=SINGLE_PACKET,
                    queue_num=(2 * c) % NQUEUES,
                )
                nc.gpsimd.dma_gather(
                    d_rows[:].rearrange("p (c d) -> p c d", d=D),
                    h[b * S : (b + 1) * S, :],
                    idx_d[:],
                    G,
                    G,
                    D,
                    single_packet=SINGLE_PACKET,
                    queue_num=(2 * c + 1) % NQUEUES,
                )
                nc.vector.tensor_tensor(
                    out=s_rows[:],
                    in0=s_rows[:],
                    in1=d_rows[:],
                    op=mybir.AluOpType.mult,
                )
                score = score_pool.tile([P, CPG], mybir.dt.float32, tag="sc")
                nc.vector.tensor_reduce(
                    out=score[:],
                    in_=s_rows[:].rearrange("p (c d) -> p c d", d=D),
                    axis=mybir.AxisListType.X,
                    op=mybir.AluOpType.add,
                )
                nc.sync.dma_start(out=out[c], in_=score[:])
    nc.compile()
    return nc


_NC_CACHE = None


def _get_nc():
    global _NC_CACHE
    if _NC_CACHE is None:
        _NC_CACHE = build_nc()
    return _NC_CACHE


def _prep_core(src_c, dst_c):
    """Bucket one core's edges. Returns (sidx, didx, pos, keep) where
    sidx/didx are the wrapped [TC, P, IW] int16 device index tensors, pos is
    each kept edge's flat position in the bucketed stream, keep the mask."""
    b = (src_c // S).astype(np.int32) * NSEG + (dst_c // S).astype(np.int32)
    # rank of each edge within its bucket, in original order
    rank = np.empty(E_PER, dtype=np.int64)
    for k in range(NBUCKET):
        m = b == k
        rank[m] = np.arange(m.sum(), dtype=np.int64)
    keep = rank < B_PAD
    pos = b.astype(np.int64) * B_PAD + rank  # valid where keep

    spad = np.zeros(NBUCKET * B_PAD, dtype=np.int16)
    dpad = np.zeros(NBUCKET * B_PAD, dtype=np.int16)
    kp = pos[keep]
    spad[kp] = (src_c[keep] % S).astype(np.int16)
    dpad[kp] = (dst_c[keep] % S).astype(np.int16)

    def wrap(arr):
        # [NBUCKET*B_PAD] -> [TC, G] -> wrapped [TC, 16, IW] -> tiled [TC, P, IW]
        a = arr.reshape(TC, IW, 16).transpose(0, 2, 1)
        return np.ascontiguousarray(np.tile(a, (1, P // 16, 1)))

    return wrap(spad), wrap(dpad), pos, keep


def build_in_maps(inputs):
    """Host-side staging: shard edges, bucket, wrap indices.
    Returns (in_maps, metas)."""
    h = np.ascontiguousarray(np.asarray(inputs["h"], dtype=np.float32))
    src = np.ascontiguousarray(np.asarray(inputs["src"]).astype(np.int32))
    dst = np.ascontiguousarray(np.asarray(inputs["dst"]).astype(np.int32))
    assert h.shape == (N, D) and src.shape == (E,) and dst.shape == (E,)
    h16 = h.astype(np.float16)

    in_maps = []
    metas = []
    for i in range(M):
        sl = slice(i * E_PER, (i + 1) * E_PER)
        sidx, didx, pos, keep = _prep_core(src[sl], dst[sl])
        in_maps.append({"h": h16, "sidx": sidx, "didx": didx})
        metas.append((pos, keep))
    return in_maps, metas


def run(inputs, trace=False, trace_kwargs=None):
    """Shard, execute on 8 cores, gather. Returns (scores[E] fp32, results)."""
    h = np.asarray(inputs["h"], dtype=np.float32)
    src = np.asarray(inputs["src"]).astype(np.int32)
    dst = np.asarray(inputs["dst"]).astype(np.int32)
    in_maps, metas = build_in_maps(inputs)

    try:
        res = run_bass_kernel_spmd(
            _get_nc(),
            in_maps,
            core_ids=list(range(M)),
            trace=trace,
            trace_kwargs=trace_kwargs or {},
        )
    except ModuleNotFoundError:
        # axon build without NTFF profiling hooks — run without trace
        res = run_bass_kernel_spmd(
            _get_nc(), in_maps, core_ids=list(range(M)), trace=False
        )

    scores = np.empty(E, dtype=np.float32)
    for i in range(M):
        sl = slice(i * E_PER, (i + 1) * E_PER)
        pos, keep = metas[i]
        out_arr = np.asarray(res.results[i]["out"], dtype=np.float32)
        # out_arr[c, p, j] is the score of bucketed position c*G + j*128 + p
        flat = out_arr.transpose(0, 2, 1).reshape(-1)
        sc = np.empty(E_PER, dtype=np.float32)
        sc[keep] = flat[pos[keep]]
        if not keep.all():  # host fallback for bucket-overflow edges
            ov = ~keep
            sc[ov] = np.einsum(
                "ed,ed->e", h[src[sl][ov]], h[dst[sl][ov]]
            ).astype(np.float32)
        scores[sl] = sc
    return scores, res


def kernel(**inputs) -> np.ndarray:
    return run(inputs)[0]

